# revision 5
# baseline (speedup 1.0000x reference)
"""Chamfer distance kernel for 8 Trainium2 NeuronCores.

Problem: x, y: [4, 8192, 3] f32 point clouds.
  D[b,i,j] = ||x[b,i] - y[b,j]||^2
  out = mean_{b,i} min_j sqrt(D) + mean_{b,j} min_i sqrt(D)

Strategy:
  - D tile = single K=5 f32 matmul on PE: [xx_i, 1, -2x_i] . [1, yy_j, y_j]
    gives xx_i + yy_j - 2 x.y directly in PSUM (f32 keeps the cancellation).
  - One DVE tensor_tensor_reduce per 4-bank PSUM span does three jobs at
    once: copies D to SBUF as fp16 (out = min(D, +inf)), and min-reduces the
    span along the free axis into a chained per-row accumulator (the row
    direction, including the final reduce). sqrt is monotone, so mins are
    taken in the squared domain.
  - Column direction: fp16 SBUF tensor_tensor min at 2x mode into a
    persistent colacc.
  - Sharding: 8 cores = 4 batches x 2 j-halves. Each core: all 64 i-chunks
    x 8 j-tiles (512 wide) of its [8192, 4096] block.
  - Host: combine per-core row/col partial mins, sqrt, mean.
"""

import sys

if "/opt/trn_rl_repo" not in sys.path:
    sys.path.insert(0, "/opt/trn_rl_repo")

import numpy as np


def _install_ntff_hook_shim():
    """The agent image's antenv lacks axon_hooks; bass_utils imports it when
    BASS_TRACE is set. Register a stand-in backed by the ctypes NTFF hook."""
    import types

    if "antenv.axon_hooks" in sys.modules:
        return
    try:
        import antenv
        from trn_agent_boot.trn_boot import _ntff_profile_via_ctypes
    except ImportError:
        return
    mod = types.ModuleType("antenv.axon_hooks")
    _hook = [None]

    def set_axon_ntff_profile_hook(h):
        _hook[0] = h

    def get_axon_ntff_profile_hook():
        if _hook[0] is None:
            try:
                _hook[0] = _ntff_profile_via_ctypes("/opt/axon/libaxon_pjrt.so")
            except Exception:
                return None
        return _hook[0]

    mod.set_axon_ntff_profile_hook = set_axon_ntff_profile_hook
    mod.get_axon_ntff_profile_hook = get_axon_ntff_profile_hook
    sys.modules["antenv.axon_hooks"] = mod
    antenv.axon_hooks = mod


_install_ntff_hook_shim()

import concourse.bacc as bacc
import concourse.bass as bass
import concourse.mybir as mybir
import concourse.tile as tile
from concourse.bass_utils import run_bass_kernel_spmd

BS = 4
N = 8192
NCHUNKS = 64           # i-chunks of 128 rows
NJT = 8                # j-tiles of 512 cols per core (half of 8192)
NSPAN = 2              # ttr spans per chunk (each spans 4 j-tiles = 2048)
JH = NJT * 512         # 4096 columns per core
N_CORES = 8
BIG = 3.0e38           # +inf stand-in (finite to be fp16/fp32-safe in min)

F32 = mybir.dt.float32
F16 = mybir.dt.float16
MIN_OP = mybir.AluOpType.min

LAST_RESULTS = None
_compiled_nc = None


def _build_program():
    nc = bacc.Bacc()

    xa = nc.declare_dram_parameter("xa", [5, N], F32, isOutput=False)
    ya = nc.declare_dram_parameter("ya", [5, JH], F32, isOutput=False)
    rowmin_out = nc.declare_dram_parameter("rowmin", [128, NCHUNKS], F32, isOutput=True)
    colmin_out = nc.declare_dram_parameter("colmin", [128, NJT, 512], F16, isOutput=True)

    COPY_FN = mybir.ActivationFunctionType.Copy

    with tile.TileContext(nc) as tc:
        with (
            tc.tile_pool(name="const", bufs=1) as const_pool,
            tc.tile_pool(name="acc", bufs=1) as acc_pool,
            tc.tile_pool(name="d16", bufs=4) as d16_pool,
            tc.tile_pool(name="psum", bufs=2, space="PSUM") as psum_pool,
        ):
            xa_sb = const_pool.tile([5, N], F32, tag="xa")
            ya_sb = const_pool.tile([5, JH], F32, tag="ya")
            nc.sync.dma_start(xa_sb[:], xa[:])
            nc.sync.dma_start(ya_sb[:], ya[:])

            colacc = acc_pool.tile([128, NJT, 512], F16, tag="colacc")
            rowmin_sb = acc_pool.tile([128, NCHUNKS], F32, tag="rowmin")

            for c in range(NCHUNKS):
                lhsT = xa_sb[:, c * 128:(c + 1) * 128]
                spans = []
                for s in range(NSPAN):
                    ps = psum_pool.tile([128, 4, 512], F32)
                    for m in range(4):
                        t = s * 4 + m
                        nc.tensor.matmul(
                            ps[:, m, :], lhsT, ya_sb[:, t * 512:(t + 1) * 512],
                            start=True, stop=True,
                        )
                    d16 = d16_pool.tile([128, 4, 512], F16)
                    if s == 0:
                        # DVE casts span 0 (1x, one wide op)
                        nc.vector.tensor_copy(d16[:], ps[:])
                    else:
                        # ACT casts span 1 (slower per element, but a free engine)
                        nc.scalar.activation(d16[:], ps[:], COPY_FN)
                    # column direction: elementwise min into persistent acc
                    span = colacc[:, s * 4:(s + 1) * 4, :]
                    if c == 0:
                        nc.vector.tensor_copy(span, d16[:])
                    else:
                        nc.vector.tensor_tensor(span, span, d16[:], MIN_OP)
                    spans.append(d16)

                # row direction: fp16 2x tt_min tree over the chunk's 4096 cols,
                # clobbering span0's buffer, then a short 1x reduce.
                r = spans[0]
                nc.vector.tensor_tensor(r[:], r[:], spans[1][:], MIN_OP)
                nc.vector.tensor_tensor(r[:, 0:2, :], r[:, 0:2, :], r[:, 2:4, :], MIN_OP)
                nc.vector.tensor_tensor(r[:, 0, :], r[:, 0, :], r[:, 1, :], MIN_OP)
                nc.vector.tensor_tensor(
                    r[:, 0, 0:256], r[:, 0, 0:256], r[:, 0, 256:512], MIN_OP
                )
                nc.vector.tensor_reduce(
                    rowmin_sb[:, c:c + 1], r[:, 0, 0:256],
                    axis=mybir.AxisListType.X, op=MIN_OP,
                )

            nc.sync.dma_start(rowmin_out[:], rowmin_sb[:])
            nc.sync.dma_start(colmin_out[:], colacc[:])

    nc.compile()
    return nc


def _augment(x, y):
    """xaugT[b]: [5, N] rows (xx, 1, -2x); yaugT[b]: [5, N] rows (1, yy, y)."""
    x = np.asarray(x, dtype=np.float32)
    y = np.asarray(y, dtype=np.float32)
    xx = (x * x).sum(-1)
    yy = (y * y).sum(-1)
    ones = np.ones_like(xx)
    xaug = np.stack([xx, ones, -2.0 * x[..., 0], -2.0 * x[..., 1], -2.0 * x[..., 2]], axis=1)
    yaug = np.stack([np.ones_like(yy), yy, y[..., 0], y[..., 1], y[..., 2]], axis=1)
    return xaug.astype(np.float32), yaug.astype(np.float32)


def kernel(x, y):
    global LAST_RESULTS, _compiled_nc

    x = np.asarray(x, dtype=np.float32)
    y = np.asarray(y, dtype=np.float32)
    bs, n, d = x.shape
    assert (bs, n, d) == (BS, N, 3), (bs, n, d)

    xaug, yaug = _augment(x, y)  # [4, 5, 8192] each

    in_maps = []
    for core in range(N_CORES):
        b, h = divmod(core, 2)
        in_maps.append({
            "xa": np.ascontiguousarray(xaug[b]),
            "ya": np.ascontiguousarray(yaug[b][:, h * JH:(h + 1) * JH]),
        })

    if _compiled_nc is None:
        _compiled_nc = _build_program()

    res = run_bass_kernel_spmd(_compiled_nc, in_maps, list(range(N_CORES)))
    LAST_RESULTS = res

    vals1_sq = np.empty((BS, N), dtype=np.float32)
    vals2_sq = np.empty((BS, N), dtype=np.float32)
    for b in range(BS):
        rm0 = res.results[2 * b]["rowmin"]      # [128, 64] f32, j-half 0
        rm1 = res.results[2 * b + 1]["rowmin"]  # [128, 64] f32, j-half 1
        rm = np.minimum(rm0, rm1)               # min over both j-halves
        # i = c*128 + p  ->  [64, 128] row-major flatten
        vals1_sq[b] = rm.T.reshape(-1)
        for h in range(2):
            ca = res.results[2 * b + h]["colmin"].astype(np.float32)  # [128, 8, 512]
            vals2_sq[b, h * JH:(h + 1) * JH] = ca.min(axis=0).reshape(-1)

    vals1 = np.sqrt(np.maximum(vals1_sq, 0.0))
    vals2 = np.sqrt(np.maximum(vals2_sq, 0.0))
    out = vals1.mean(axis=1).mean() + vals2.mean(axis=1).mean()
    return np.float32(out)


# revision 6
# speedup vs baseline: 1.7970x; 1.7970x over previous
"""Chamfer distance kernel for 8 Trainium2 NeuronCores.

Problem: x, y: [4, 8192, 3] f32 point clouds.
  D[b,i,j] = ||x[b,i] - y[b,j]||^2
  out = mean_{b,i} min_j sqrt(D) + mean_{b,j} min_i sqrt(D)

Strategy:
  - D tile = single K=5 f32 matmul on PE: [xx_i, 1, -2x_i] . [1, yy_j, y_j]
    gives xx_i + yy_j - 2 x.y directly in PSUM (f32 keeps the cancellation).
  - One DVE tensor_tensor_reduce per 4-bank PSUM span does three jobs at
    once: copies D to SBUF as fp16 (out = min(D, +inf)), and min-reduces the
    span along the free axis into a chained per-row accumulator (the row
    direction, including the final reduce). sqrt is monotone, so mins are
    taken in the squared domain.
  - Column direction: fp16 SBUF tensor_tensor min at 2x mode into a
    persistent colacc.
  - Sharding: 8 cores = 4 batches x 2 j-halves. Each core: all 64 i-chunks
    x 8 j-tiles (512 wide) of its [8192, 4096] block.
  - Host: combine per-core row/col partial mins, sqrt, mean.
"""

import sys

if "/opt/trn_rl_repo" not in sys.path:
    sys.path.insert(0, "/opt/trn_rl_repo")

import numpy as np


def _install_ntff_hook_shim():
    """The agent image's antenv lacks axon_hooks; bass_utils imports it when
    BASS_TRACE is set. Register a stand-in backed by the ctypes NTFF hook."""
    import types

    if "antenv.axon_hooks" in sys.modules:
        return
    try:
        import antenv
        from trn_agent_boot.trn_boot import _ntff_profile_via_ctypes
    except ImportError:
        return
    mod = types.ModuleType("antenv.axon_hooks")
    _hook = [None]

    def set_axon_ntff_profile_hook(h):
        _hook[0] = h

    def get_axon_ntff_profile_hook():
        if _hook[0] is None:
            try:
                _hook[0] = _ntff_profile_via_ctypes("/opt/axon/libaxon_pjrt.so")
            except Exception:
                return None
        return _hook[0]

    mod.set_axon_ntff_profile_hook = set_axon_ntff_profile_hook
    mod.get_axon_ntff_profile_hook = get_axon_ntff_profile_hook
    sys.modules["antenv.axon_hooks"] = mod
    antenv.axon_hooks = mod


_install_ntff_hook_shim()

import concourse.bacc as bacc
import concourse.bass as bass
import concourse.mybir as mybir
import concourse.tile as tile
from concourse.bass_utils import run_bass_kernel_spmd

BS = 4
N = 8192
NCHUNKS = 64           # i-chunks of 128 rows
NJT = 8                # j-tiles of 512 cols per core (half of 8192)
NSPAN = 2              # ttr spans per chunk (each spans 4 j-tiles = 2048)
JH = NJT * 512         # 4096 columns per core
N_CORES = 8
BIG = 3.0e38           # +inf stand-in (finite to be fp16/fp32-safe in min)

F32 = mybir.dt.float32
F16 = mybir.dt.float16
MIN_OP = mybir.AluOpType.min

LAST_RESULTS = None
_compiled_nc = None


def _build_program():
    nc = bacc.Bacc()

    xa = nc.declare_dram_parameter("xa", [5, N], F32, isOutput=False)
    ya = nc.declare_dram_parameter("ya", [5, JH], F32, isOutput=False)
    rowmin_out = nc.declare_dram_parameter("rowmin", [128, NCHUNKS], F32, isOutput=True)
    colmin_out = nc.declare_dram_parameter("colmin", [128, NJT, 512], F16, isOutput=True)

    COPY_FN = mybir.ActivationFunctionType.Copy

    with tile.TileContext(nc) as tc:
        with (
            tc.tile_pool(name="const", bufs=1) as const_pool,
            tc.tile_pool(name="acc", bufs=1) as acc_pool,
            tc.tile_pool(name="d16", bufs=4) as d16_pool,
            tc.tile_pool(name="psum", bufs=2, space="PSUM") as psum_pool,
        ):
            # xa/ya replicated at partition offsets 0/32/64/96 so four K=5
            # matmuls run concurrently in distinct PE row-groups (4x PE).
            xa_sb = const_pool.tile([101, N], F32, tag="xa")
            ya_sb = const_pool.tile([101, JH], F32, tag="ya")
            for m in range(4):
                nc.sync.dma_start(xa_sb[32 * m:32 * m + 5, :], xa[:])
                nc.sync.dma_start(ya_sb[32 * m:32 * m + 5, :], ya[:])

            colacc = acc_pool.tile([128, NJT, 512], F16, tag="colacc")
            rowmin_sb = acc_pool.tile([128, NCHUNKS], F32, tag="rowmin")

            for c in range(NCHUNKS):
                spans = []
                for s in range(NSPAN):
                    ps = psum_pool.tile([128, 4, 512], F32)
                    for m in range(4):
                        t = s * 4 + m
                        nc.tensor.matmul(
                            ps[:, m, :],
                            xa_sb[32 * m:32 * m + 5, c * 128:(c + 1) * 128],
                            ya_sb[32 * m:32 * m + 5, t * 512:(t + 1) * 512],
                            start=True, stop=True,
                            tile_position=(32 * m, 0),
                        )
                    d16 = d16_pool.tile([128, 4, 512], F16)
                    if s == 0:
                        # DVE casts span 0 (1x, one wide op)
                        nc.vector.tensor_copy(d16[:], ps[:])
                    else:
                        # ACT casts span 1 (slower per element, but a free engine)
                        nc.scalar.activation(d16[:], ps[:], COPY_FN)
                    # column direction: elementwise min into persistent acc
                    span = colacc[:, s * 4:(s + 1) * 4, :]
                    if c == 0:
                        nc.vector.tensor_copy(span, d16[:])
                    else:
                        nc.vector.tensor_tensor(span, span, d16[:], MIN_OP)
                    spans.append(d16)

                # row direction: fp16 2x tt_min tree over the chunk's 4096 cols,
                # clobbering span0's buffer, then a short 1x reduce.
                r = spans[0]
                nc.vector.tensor_tensor(r[:], r[:], spans[1][:], MIN_OP)
                nc.vector.tensor_tensor(r[:, 0:2, :], r[:, 0:2, :], r[:, 2:4, :], MIN_OP)
                nc.vector.tensor_tensor(r[:, 0, :], r[:, 0, :], r[:, 1, :], MIN_OP)
                nc.vector.tensor_tensor(
                    r[:, 0, 0:256], r[:, 0, 0:256], r[:, 0, 256:512], MIN_OP
                )
                nc.vector.tensor_reduce(
                    rowmin_sb[:, c:c + 1], r[:, 0, 0:256],
                    axis=mybir.AxisListType.X, op=MIN_OP,
                )

            nc.sync.dma_start(rowmin_out[:], rowmin_sb[:])
            nc.sync.dma_start(colmin_out[:], colacc[:])

    nc.compile()
    return nc


def _augment(x, y):
    """xaugT[b]: [5, N] rows (xx, 1, -2x); yaugT[b]: [5, N] rows (1, yy, y)."""
    x = np.asarray(x, dtype=np.float32)
    y = np.asarray(y, dtype=np.float32)
    xx = (x * x).sum(-1)
    yy = (y * y).sum(-1)
    ones = np.ones_like(xx)
    xaug = np.stack([xx, ones, -2.0 * x[..., 0], -2.0 * x[..., 1], -2.0 * x[..., 2]], axis=1)
    yaug = np.stack([np.ones_like(yy), yy, y[..., 0], y[..., 1], y[..., 2]], axis=1)
    return xaug.astype(np.float32), yaug.astype(np.float32)


def kernel(x, y):
    global LAST_RESULTS, _compiled_nc

    x = np.asarray(x, dtype=np.float32)
    y = np.asarray(y, dtype=np.float32)
    bs, n, d = x.shape
    assert (bs, n, d) == (BS, N, 3), (bs, n, d)

    xaug, yaug = _augment(x, y)  # [4, 5, 8192] each

    in_maps = []
    for core in range(N_CORES):
        b, h = divmod(core, 2)
        in_maps.append({
            "xa": np.ascontiguousarray(xaug[b]),
            "ya": np.ascontiguousarray(yaug[b][:, h * JH:(h + 1) * JH]),
        })

    if _compiled_nc is None:
        _compiled_nc = _build_program()

    res = run_bass_kernel_spmd(_compiled_nc, in_maps, list(range(N_CORES)))
    LAST_RESULTS = res

    vals1_sq = np.empty((BS, N), dtype=np.float32)
    vals2_sq = np.empty((BS, N), dtype=np.float32)
    for b in range(BS):
        rm0 = res.results[2 * b]["rowmin"]      # [128, 64] f32, j-half 0
        rm1 = res.results[2 * b + 1]["rowmin"]  # [128, 64] f32, j-half 1
        rm = np.minimum(rm0, rm1)               # min over both j-halves
        # i = c*128 + p  ->  [64, 128] row-major flatten
        vals1_sq[b] = rm.T.reshape(-1)
        for h in range(2):
            ca = res.results[2 * b + h]["colmin"].astype(np.float32)  # [128, 8, 512]
            vals2_sq[b, h * JH:(h + 1) * JH] = ca.min(axis=0).reshape(-1)

    vals1 = np.sqrt(np.maximum(vals1_sq, 0.0))
    vals2 = np.sqrt(np.maximum(vals2_sq, 0.0))
    out = vals1.mean(axis=1).mean() + vals2.mean(axis=1).mean()
    return np.float32(out)


# revision 7
# speedup vs baseline: 2.3626x; 1.3148x over previous
"""Chamfer distance kernel for 8 Trainium2 NeuronCores.

Problem: x, y: [4, 8192, 3] f32 point clouds.
  D[b,i,j] = ||x[b,i] - y[b,j]||^2
  out = mean_{b,i} min_j sqrt(D) + mean_{b,j} min_i sqrt(D)

Strategy:
  - D tile = single K=5 f32 matmul on PE: [xx_i, 1, -2x_i] . [1, yy_j, y_j]
    gives xx_i + yy_j - 2 x.y directly in PSUM (f32 keeps the cancellation).
  - One DVE tensor_tensor_reduce per 4-bank PSUM span does three jobs at
    once: copies D to SBUF as fp16 (out = min(D, +inf)), and min-reduces the
    span along the free axis into a chained per-row accumulator (the row
    direction, including the final reduce). sqrt is monotone, so mins are
    taken in the squared domain.
  - Column direction: fp16 SBUF tensor_tensor min at 2x mode into a
    persistent colacc.
  - Sharding: 8 cores = 4 batches x 2 j-halves. Each core: all 64 i-chunks
    x 8 j-tiles (512 wide) of its [8192, 4096] block.
  - Host: combine per-core row/col partial mins, sqrt, mean.
"""

import sys

if "/opt/trn_rl_repo" not in sys.path:
    sys.path.insert(0, "/opt/trn_rl_repo")

import numpy as np


def _install_ntff_hook_shim():
    """The agent image's antenv lacks axon_hooks; bass_utils imports it when
    BASS_TRACE is set. Register a stand-in backed by the ctypes NTFF hook."""
    import types

    if "antenv.axon_hooks" in sys.modules:
        return
    try:
        import antenv
        from trn_agent_boot.trn_boot import _ntff_profile_via_ctypes
    except ImportError:
        return
    mod = types.ModuleType("antenv.axon_hooks")
    _hook = [None]

    def set_axon_ntff_profile_hook(h):
        _hook[0] = h

    def get_axon_ntff_profile_hook():
        if _hook[0] is None:
            try:
                _hook[0] = _ntff_profile_via_ctypes("/opt/axon/libaxon_pjrt.so")
            except Exception:
                return None
        return _hook[0]

    mod.set_axon_ntff_profile_hook = set_axon_ntff_profile_hook
    mod.get_axon_ntff_profile_hook = get_axon_ntff_profile_hook
    sys.modules["antenv.axon_hooks"] = mod
    antenv.axon_hooks = mod


_install_ntff_hook_shim()

import concourse.bacc as bacc
import concourse.bass as bass
import concourse.mybir as mybir
import concourse.tile as tile
from concourse.bass_utils import run_bass_kernel_spmd

BS = 4
N = 8192
NCHUNKS = 64           # i-chunks of 128 rows
NJT = 8                # j-tiles of 512 cols per core (half of 8192)
NSPAN = 2              # ttr spans per chunk (each spans 4 j-tiles = 2048)
JH = NJT * 512         # 4096 columns per core
N_CORES = 8
BIG = 3.0e38           # +inf stand-in (finite to be fp16/fp32-safe in min)

F32 = mybir.dt.float32
F16 = mybir.dt.float16
MIN_OP = mybir.AluOpType.min

LAST_RESULTS = None
_compiled_nc = None


def _build_program():
    nc = bacc.Bacc()

    xa = nc.declare_dram_parameter("xa", [5, N], F32, isOutput=False)
    ya = nc.declare_dram_parameter("ya", [5, JH], F32, isOutput=False)
    rowmin_out = nc.declare_dram_parameter("rowmin", [128, NCHUNKS], F32, isOutput=True)
    colmin_out = nc.declare_dram_parameter("colmin", [128, NJT, 512], F16, isOutput=True)

    COPY_FN = mybir.ActivationFunctionType.Copy

    with tile.TileContext(nc) as tc:
        with (
            tc.tile_pool(name="const", bufs=1) as const_pool,
            tc.tile_pool(name="acc", bufs=1) as acc_pool,
            tc.tile_pool(name="d16", bufs=4) as d16_pool,
            tc.tile_pool(name="psum", bufs=2, space="PSUM") as psum_pool,
        ):
            # xa/ya replicated at partition offsets 0/32/64/96 so four K=5
            # matmuls run concurrently in distinct PE row-groups (4x PE).
            xa_sb = const_pool.tile([101, N], F32, tag="xa")
            ya_sb = const_pool.tile([101, JH], F32, tag="ya")
            for m in range(4):
                nc.sync.dma_start(xa_sb[32 * m:32 * m + 5, :], xa[:])
                nc.sync.dma_start(ya_sb[32 * m:32 * m + 5, :], ya[:])

            colacc = acc_pool.tile([128, NJT, 512], F16, tag="colacc")
            rowmin_sb = acc_pool.tile([128, NCHUNKS], F32, tag="rowmin")

            for c in range(NCHUNKS):
                spans = []
                for s in range(NSPAN):
                    ps = psum_pool.tile([128, 4, 512], F32)
                    for m in range(4):
                        t = s * 4 + m
                        nc.tensor.matmul(
                            ps[:, m, :],
                            xa_sb[32 * m:32 * m + 5, c * 128:(c + 1) * 128],
                            ya_sb[32 * m:32 * m + 5, t * 512:(t + 1) * 512],
                            start=True, stop=True,
                            tile_position=(32 * m, 0),
                        )
                    d16 = d16_pool.tile([128, 4, 512], F16)
                    # ACT casts all spans PSUM->SBUF fp16; DVE only does mins
                    nc.scalar.activation(d16[:], ps[:], COPY_FN)
                    # column direction: elementwise min into persistent acc
                    span = colacc[:, s * 4:(s + 1) * 4, :]
                    if c == 0:
                        nc.vector.tensor_copy(span, d16[:])
                    else:
                        nc.vector.tensor_tensor(span, span, d16[:], MIN_OP)
                    spans.append(d16)

                # row direction: fp16 2x tt_min tree over the chunk's 4096 cols,
                # clobbering span0's buffer, then a short 1x reduce.
                r = spans[0]
                nc.vector.tensor_tensor(r[:], r[:], spans[1][:], MIN_OP)
                nc.vector.tensor_tensor(r[:, 0:2, :], r[:, 0:2, :], r[:, 2:4, :], MIN_OP)
                nc.vector.tensor_tensor(r[:, 0, :], r[:, 0, :], r[:, 1, :], MIN_OP)
                nc.vector.tensor_tensor(
                    r[:, 0, 0:256], r[:, 0, 0:256], r[:, 0, 256:512], MIN_OP
                )
                nc.vector.tensor_reduce(
                    rowmin_sb[:, c:c + 1], r[:, 0, 0:256],
                    axis=mybir.AxisListType.X, op=MIN_OP,
                )

            nc.sync.dma_start(rowmin_out[:], rowmin_sb[:])
            nc.sync.dma_start(colmin_out[:], colacc[:])

    nc.compile()
    return nc


def _augment(x, y):
    """xaugT[b]: [5, N] rows (xx, 1, -2x); yaugT[b]: [5, N] rows (1, yy, y)."""
    x = np.asarray(x, dtype=np.float32)
    y = np.asarray(y, dtype=np.float32)
    xx = (x * x).sum(-1)
    yy = (y * y).sum(-1)
    ones = np.ones_like(xx)
    xaug = np.stack([xx, ones, -2.0 * x[..., 0], -2.0 * x[..., 1], -2.0 * x[..., 2]], axis=1)
    yaug = np.stack([np.ones_like(yy), yy, y[..., 0], y[..., 1], y[..., 2]], axis=1)
    return xaug.astype(np.float32), yaug.astype(np.float32)


def kernel(x, y):
    global LAST_RESULTS, _compiled_nc

    x = np.asarray(x, dtype=np.float32)
    y = np.asarray(y, dtype=np.float32)
    bs, n, d = x.shape
    assert (bs, n, d) == (BS, N, 3), (bs, n, d)

    xaug, yaug = _augment(x, y)  # [4, 5, 8192] each

    in_maps = []
    for core in range(N_CORES):
        b, h = divmod(core, 2)
        in_maps.append({
            "xa": np.ascontiguousarray(xaug[b]),
            "ya": np.ascontiguousarray(yaug[b][:, h * JH:(h + 1) * JH]),
        })

    if _compiled_nc is None:
        _compiled_nc = _build_program()

    res = run_bass_kernel_spmd(_compiled_nc, in_maps, list(range(N_CORES)))
    LAST_RESULTS = res

    vals1_sq = np.empty((BS, N), dtype=np.float32)
    vals2_sq = np.empty((BS, N), dtype=np.float32)
    for b in range(BS):
        rm0 = res.results[2 * b]["rowmin"]      # [128, 64] f32, j-half 0
        rm1 = res.results[2 * b + 1]["rowmin"]  # [128, 64] f32, j-half 1
        rm = np.minimum(rm0, rm1)               # min over both j-halves
        # i = c*128 + p  ->  [64, 128] row-major flatten
        vals1_sq[b] = rm.T.reshape(-1)
        for h in range(2):
            ca = res.results[2 * b + h]["colmin"].astype(np.float32)  # [128, 8, 512]
            vals2_sq[b, h * JH:(h + 1) * JH] = ca.min(axis=0).reshape(-1)

    vals1 = np.sqrt(np.maximum(vals1_sq, 0.0))
    vals2 = np.sqrt(np.maximum(vals2_sq, 0.0))
    out = vals1.mean(axis=1).mean() + vals2.mean(axis=1).mean()
    return np.float32(out)
